# revision 7
# baseline (speedup 1.0000x reference)
"""Trainium2 Bass kernel for nn_CrossAttFuseMod (8 NeuronCores, SPMD).

Sharding: data-parallel over the batch dim B=2048 (256 rows/core).  The
mix/cross-attention stack is degenerate (per-sample seq len 1 => softmax==1),
so the whole pre-encoder is folded on the host into two linear maps.  The
TransformerEncoder self-attention (seq=B, batch=1) gathers K^T and V across
cores with AllGather; everything else is local.  Activations are kept
feature-major ("transposed", [feat, rows]) on-chip so no matmul ever needs an
on-chip transpose; partition-axis reductions (softmax denom, LayerNorm stats)
are done with ones-vector matmuls on the tensor engine.  ScalarE stays inside
one LUT table set (exp/ln + fillers): rsqrt = exp(-0.5*ln(v)), 1/S = exp(-ln S).

dtypes: heavy matmuls run bf16 x bf16 -> fp32 PSUM (1 cycle/row on the PE and
half the weight DMA); the residual stream, LayerNorm stats, softmax sums and
all outputs stay fp32.  Measured vs the fp32 reference this lands ~3e-3 max
relative error.
"""

import numpy as np
import ml_dtypes

BF16NP = ml_dtypes.bfloat16

NCORES = 8
B = 2048
E = 2048          # encoder d_model
R = B // NCORES   # rows per core = 256
P = 128           # SBUF partitions
KT = E // P       # 16 k-tiles over the model dim
HN = 8            # heads
HD = E // HN      # 256 head dim
DFF = 2048
HID = 1024

_CACHE = {}


def _pmajor(v):
    # bias vector [F] -> [128, F//128] with (p, m) = v[m*128 + p]
    v = np.ascontiguousarray(np.asarray(v, np.float32))
    return np.ascontiguousarray(v.reshape(-1, P).T)


def _bf(a):
    return np.ascontiguousarray(np.asarray(a, np.float32).astype(BF16NP))


def _fold_params(params):
    """Fold the (purely linear) pre-encoder into x = [T@maW.T+mac | G@mbG.T+mbc]."""
    f8 = np.float64
    g = lambda a: np.asarray(a, f8)
    A, a = g(params['te1_W']), g(params['te1_b'])
    Bm, b = g(params['ge1_W']), g(params['ge1_b'])
    Wt, Gt, ct = A, np.zeros((HID, E)), a
    Wg, Gg, cg = np.zeros((HID, E)), Bm, b
    for pa, pb in zip(params['mix_a'], params['mix_b']):
        Ma = g(pa['Wo']) @ g(pa['Wv'])
        Mb = g(pb['Wo']) @ g(pb['Wv'])
        ba, bb = g(pa['bo']), g(pb['bo'])
        nWt, nGt, nct = Ma @ Wg, Ma @ Gg, Ma @ cg + ba
        nWg, nGg, ncg = Mb @ Wt, Mb @ Gt, Mb @ ct + bb
        Wt, Gt, ct, Wg, Gg, cg = nWt, nGt, nct, nWg, nGg, ncg
    C, c = g(params['te2_W']), g(params['te2_b'])
    D, d = g(params['ge2_W']), g(params['ge2_b'])
    maW, maG, mac = C @ Wt, C @ Gt, C @ ct + c
    mbW, mbG, mbc = D @ Wg, D @ Gg, D @ cg + d
    return (maW, maG, mac), (mbW, mbG, mbc)


def _host_prep(params):
    """All host-side weight prep; returns dict of np arrays shipped to device."""
    (maW, maG, mac), (mbW, mbG, mbc) = _fold_params(params)
    t = {}
    t['PT'] = _bf(maW.T)   # [2048, 1024]
    t['QT'] = _bf(mbG.T)   # [2048, 1024]
    t['pqbP'] = _pmajor(np.concatenate([mac, mbc]).astype(np.float32))  # [128, 16]
    # with an even mix count, ma depends only on text and mb only on graph
    t['cross'] = bool(np.abs(maG).max() > 0 or np.abs(mbW).max() > 0)
    for l, lp in enumerate(params['enc']):
        t[f'winT{l}'] = _bf(np.asarray(lp['Win'], np.float32).T)  # [2048, 6144]
        t[f'binP{l}'] = _pmajor(lp['bin'])                        # [128, 48]
        t[f'woT{l}'] = _bf(np.asarray(lp['Wo'], np.float32).T)
        t[f'boP{l}'] = _pmajor(lp['bo'])
        t[f'w1T{l}'] = _bf(np.asarray(lp['W1'], np.float32).T)
        t[f'c1P{l}'] = _pmajor(lp['c1'])
        t[f'w2T{l}'] = _bf(np.asarray(lp['W2'], np.float32).T)
        t[f'c2P{l}'] = _pmajor(lp['c2'])
        t[f'g1P{l}'] = _pmajor(lp['g1'])
        t[f'b1P{l}'] = _pmajor(lp['b1'])
        t[f'g2P{l}'] = _pmajor(lp['g2'])
        t[f'b2P{l}'] = _pmajor(lp['b2'])
    t['recWT'] = _bf(np.asarray(params['rec_W'], np.float32).T)   # [2048, 2048]
    t['recb'] = np.ascontiguousarray(np.asarray(params['rec_b'], np.float32).reshape(1, -1))
    return t


def _build_program(n_layers=2):
    import concourse.bacc as bacc
    import concourse.mybir as mybir
    import concourse.tile as tile

    dt = mybir.dt
    F32 = dt.float32
    BF16 = dt.bfloat16
    AF = mybir.ActivationFunctionType
    ALU = mybir.AluOpType
    RG = [list(range(NCORES))]

    nc = bacc.Bacc("TRN2", target_bir_lowering=False, debug=False,
                   num_devices=NCORES, name="crossattfuse")

    ins = {}

    def inp(name, shape, dtp=F32):
        ins[name] = nc.dram_tensor(name, list(shape), dtp, kind="ExternalInput")
        return ins[name]

    tT = inp('tT', (E, R), BF16)
    gT = inp('gT', (E, R), BF16)
    PT = inp('PT', (E, HID), BF16)
    QT = inp('QT', (E, HID), BF16)
    inp('pqbP', (P, 16))
    for l in range(n_layers):
        inp(f'winT{l}', (E, 3 * E), BF16)
        inp(f'binP{l}', (P, 48))
        inp(f'woT{l}', (E, E), BF16)
        inp(f'boP{l}', (P, 16))
        inp(f'w1T{l}', (E, DFF), BF16)
        inp(f'c1P{l}', (P, DFF // P))
        inp(f'w2T{l}', (DFF, E), BF16)
        inp(f'c2P{l}', (P, 16))
        inp(f'g1P{l}', (P, 16))
        inp(f'b1P{l}', (P, 16))
        inp(f'g2P{l}', (P, 16))
        inp(f'b2P{l}', (P, 16))
    recWT = inp('recWT', (E, E), BF16)
    recb = inp('recb', (1, E))

    out_x = nc.dram_tensor('out_x', [R, E], F32, kind="ExternalOutput")
    out_rec = nc.dram_tensor('out_rec', [R, E], F32, kind="ExternalOutput")

    ident_dram = nc.inline_tensor(np.eye(P, dtype=np.float32), name="ident128")

    with tile.TileContext(nc) as tc:
        with (
            tc.tile_pool(name="const", bufs=1) as pc,
            tc.tile_pool(name="act", bufs=56) as pa,       # fp32 [128,256] tiles
            tc.tile_pool(name="actb", bufs=56) as pb,      # bf16 [128,256] tiles
            tc.tile_pool(name="wslab", bufs=4) as pw,
            tc.tile_pool(name="misc", bufs=2) as pm,
            tc.tile_pool(name="big", bufs=2) as pbig,
            tc.tile_pool(name="psum", bufs=8, space="PSUM") as pp,
            tc.tile_pool(name="dram", bufs=1, space="DRAM") as pd,
        ):
            def atile():
                return pa.tile([P, R], F32, tag="a", name="a")

            def btile():
                return pb.tile([P, R], BF16, tag="ab", name="ab")

            # ---- constants ----
            ones_col = pc.tile([P, 1], F32, tag="ones_col", name="ones_col")
            nc.vector.memset(ones_col[:], 1.0)
            ones_colb = pc.tile([P, 1], BF16, tag="ones_colb", name="ones_colb")
            nc.vector.memset(ones_colb[:], 1.0)
            ones_row = pc.tile([1, P], F32, tag="ones_row", name="ones_row")
            nc.vector.memset(ones_row[:], 1.0)
            ident = pc.tile([P, P], F32, tag="ident", name="ident")
            nc.sync.dma_start(out=ident[:], in_=ident_dram.ap())

            def load_pm(name, cols):
                tl = pc.tile([P, cols], F32, tag=name)
                nc.sync.dma_start(out=tl[:], in_=ins[name].ap())
                return tl

            pqb = load_pm('pqbP', 16)
            lw = []
            for l in range(n_layers):
                lw.append({k: load_pm(f'{k}{l}', 48 if k == 'binP' else 16)
                           for k in ('binP', 'boP', 'c1P', 'c2P', 'g1P', 'b1P', 'g2P', 'b2P')})

            # rec_b broadcast tile [128, 2048] (k=1 ones matmuls, fp32)
            recb_sb = pc.tile([1, E], F32, tag="recb_sb", name="recb_sb")
            nc.sync.dma_start(out=recb_sb[:], in_=recb.ap())
            recb_bc = pc.tile([P, E], F32, tag="recb_bc", name="recb_bc")
            for n in range(E // 512):
                ps = pp.tile([P, 512], F32, tag="mm", name="mm")
                nc.tensor.matmul(ps[:], ones_row[:], recb_sb[:, n * 512:(n + 1) * 512])
                nc.scalar.copy(recb_bc[:, n * 512:(n + 1) * 512], ps[:])

            # ---- generic bf16 matmul sweep: out[m] = sum_k Wslab(k,m).T @ rhs[k] ----
            def mm_sweep(wdram, rhs_tiles, n_mt, evac, wcol0=0, group=4):
                n_k = len(rhs_tiles)
                for m0 in range(0, n_mt, group):
                    gsz = min(group, n_mt - m0)
                    psl = [pp.tile([P, R], F32, tag="mm", name="mm") for _ in range(gsz)]
                    for k in range(n_k):
                        slab = pw.tile([P, group * P], BF16, tag="wslab", name="wslab")
                        c0 = wcol0 + m0 * P
                        nc.sync.dma_start(
                            out=slab[:, :gsz * P],
                            in_=wdram.ap()[k * P:(k + 1) * P, c0:c0 + gsz * P])
                        for j in range(gsz):
                            nc.tensor.matmul(
                                psl[j][:], slab[:, j * P:(j + 1) * P], rhs_tiles[k][:],
                                start=(k == 0), stop=(k == n_k - 1))
                    for j in range(gsz):
                        evac(m0 + j, psl[j])

            def bshadow(x_tiles):
                xb = []
                for k in range(KT):
                    xbk = btile()
                    nc.vector.tensor_copy(xbk[:], x_tiles[k][:])
                    xb.append(xbk)
                return xb

            # ---- phase 1: folded pre-encoder ----
            tin = [btile() for _ in range(KT)]
            gin = [btile() for _ in range(KT)]
            for k in range(KT):
                nc.sync.dma_start(out=tin[k][:], in_=tT.ap()[k * P:(k + 1) * P, :])
                nc.sync.dma_start(out=gin[k][:], in_=gT.ap()[k * P:(k + 1) * P, :])

            x_cur = [atile() for _ in range(KT)]

            def evac_x(base):
                def f(m, ps):
                    nc.scalar.activation(x_cur[base + m][:], ps[:], AF.Identity,
                                         bias=pqb[:, base + m:base + m + 1], scale=1.0)
                return f
            mm_sweep(PT, tin, HID // P, evac_x(0))
            mm_sweep(QT, gin, HID // P, evac_x(8))

            # ---- encoder layers ----
            for l in range(n_layers):
                w = lw[l]
                winT = ins[f'winT{l}']
                xb_cur = bshadow(x_cur)
                kb = pd.tile([E, R], BF16, tag=f"kb{l}", name="kb")
                vb = pd.tile([R, E], BF16, tag=f"vb{l}", name="vb")
                agk = pd.tile([NCORES * E, R], BF16, tag=f"agk{l}", name="agk",
                              addr_space="Shared")
                agv = pd.tile([B, E], BF16, tag=f"agv{l}", name="agv",
                              addr_space="Shared")

                # --- K^T (winT cols 2048:4096): evacuate bf16 + bounce, AllGather ---
                def evac_k(m, ps):
                    kt = btile()
                    nc.scalar.activation(kt[:], ps[:], AF.Identity,
                                         bias=w['binP'][:, 16 + m:17 + m], scale=1.0)
                    nc.sync.dma_start(out=kb[m * P:(m + 1) * P, :], in_=kt[:])
                mm_sweep(winT, xb_cur, KT, evac_k, wcol0=E)
                nc.gpsimd.collective_compute(
                    "AllGather", ALU.bypass, replica_groups=RG,
                    ins=[kb.opt()], outs=[agk.opt()])

                # --- V row-major (no bias; folded into oT evac): ---
                for mi in range(2):
                    for dn in range(E // 512):
                        psv = pp.tile([P, 512], F32, tag="mm", name="mm")
                        for k in range(KT):
                            slab = pw.tile([P, 512], BF16, tag="wslab", name="wslab")
                            nc.sync.dma_start(
                                out=slab[:],
                                in_=winT.ap()[k * P:(k + 1) * P,
                                              2 * E + dn * 512:2 * E + (dn + 1) * 512])
                            nc.tensor.matmul(
                                psv[:], xb_cur[k][:, mi * P:(mi + 1) * P], slab[:],
                                start=(k == 0), stop=(k == KT - 1))
                        vsb = pm.tile([P, 512], BF16, tag="vev", bufs=4, name="vev")
                        nc.scalar.copy(vsb[:], psv[:])
                        nc.sync.dma_start(
                            out=vb[mi * P:(mi + 1) * P, dn * 512:(dn + 1) * 512],
                            in_=vsb[:])
                nc.gpsimd.collective_compute(
                    "AllGather", ALU.bypass, replica_groups=RG,
                    ins=[vb.opt()], outs=[agv.opt()])

                # --- Q^T (winT cols 0:2048) ---
                qT = [btile() for _ in range(KT)]

                def evac_q(m, ps):
                    nc.scalar.activation(qT[m][:], ps[:], AF.Identity,
                                         bias=w['binP'][:, m:m + 1], scale=1.0)
                mm_sweep(winT, xb_cur, KT, evac_q, wcol0=0)

                # --- attention: 4 waves x 2 heads; scores computed t-major (sT) ---
                oT = [btile() for _ in range(KT)]
                inv_sqrt_hd = float(1.0 / np.sqrt(HD))
                for wv in range(4):
                    h0 = 2 * wv
                    S_ps = pp.tile([1, 512], F32, tag="mm", name="mm")
                    o_ps = [pp.tile([P, R], F32, tag="mm", name="mm") for _ in range(4)]
                    for t in range(KT):
                        tg = t * P
                        rrank, tloc = tg // R, tg % R
                        base = rrank * E + h0 * HD
                        kslab = pbig.tile([P, 512], BF16, tag="kslab", name="kslab")
                        nc.sync.dma_start(
                            out=kslab.rearrange("p (blk t) -> p blk t", blk=4),
                            in_=agk[base:base + 512, tloc:tloc + P]
                                .rearrange("(blk p) t -> p blk t", p=P))
                        vslab = pbig.tile([P, 512], BF16, tag="vslab", name="vslab")
                        nc.sync.dma_start(
                            out=vslab[:],
                            in_=agv[tg:tg + P, h0 * HD:h0 * HD + 512])
                        aT = pbig.tile([P, 512], BF16, tag="aT", name="aT")
                        for hh in range(2):
                            s_ps = pp.tile([P, R], F32, tag="mm", name="mm")
                            for dd in range(2):
                                nc.tensor.matmul(
                                    s_ps[:],
                                    kslab[:, (2 * hh + dd) * P:(2 * hh + dd + 1) * P],
                                    qT[2 * (h0 + hh) + dd][:],
                                    start=(dd == 0), stop=(dd == 1))
                            nc.scalar.activation(aT[:, hh * R:(hh + 1) * R], s_ps[:],
                                                 AF.Exp, scale=inv_sqrt_hd)
                        nc.tensor.matmul(S_ps[:], ones_colb[:], aT[:],
                                         start=(t == 0), stop=(t == KT - 1))
                        for j in range(4):
                            hh = j // 2
                            nc.tensor.matmul(
                                o_ps[j][:], vslab[:, j * P:(j + 1) * P],
                                aT[:, hh * R:(hh + 1) * R],
                                start=(t == 0), stop=(t == KT - 1))
                    # normalize: oT = o_ps * (1/S) + bv   (1/S = exp(-ln(S)))
                    lnS = pm.tile([1, 512], F32, tag="lnS", name="lnS")
                    nc.scalar.activation(lnS[:], S_ps[:], AF.Ln)
                    rS = pm.tile([1, 512], F32, tag="rS", name="rS")
                    nc.scalar.activation(rS[:], lnS[:], AF.Exp, scale=-1.0)
                    bc_ps = pp.tile([P, 512], F32, tag="mm", name="mm")
                    nc.tensor.matmul(bc_ps[:], ones_row[:], rS[:])
                    bc_sb = pm.tile([P, 512], F32, tag="bcS", name="bcS")
                    nc.scalar.copy(bc_sb[:], bc_ps[:])
                    for j in range(4):
                        hh, mi = j // 2, j % 2
                        osc = pm.tile([P, R], F32, tag="osc", bufs=4, name="osc")
                        nc.vector.tensor_tensor(osc[:], o_ps[j][:],
                                                bc_sb[:, hh * R:(hh + 1) * R], ALU.mult)
                        nc.scalar.activation(
                            oT[2 * (h0 + hh) + mi][:], osc[:], AF.Identity,
                            bias=w['binP'][:, 32 + 2 * (h0 + hh) + mi:
                                           33 + 2 * (h0 + hh) + mi], scale=1.0)

                # --- LayerNorm (partition-axis stats via fp32 ones-matmuls) ---
                def layer_norm(u_tiles, gP, bP, out_tiles):
                    sum_ps = pp.tile([1, R], F32, tag="mm", name="mm")
                    sq_ps = pp.tile([1, R], F32, tag="mm", name="mm")
                    for k in range(KT):
                        nc.tensor.matmul(sum_ps[:], ones_col[:], u_tiles[k][:],
                                         start=(k == 0), stop=(k == KT - 1))
                        sq = pm.tile([P, R], F32, tag="sq", bufs=3, name="sq")
                        nc.scalar.square(sq[:], u_tiles[k][:])
                        nc.tensor.matmul(sq_ps[:], ones_col[:], sq[:],
                                         start=(k == 0), stop=(k == KT - 1))
                    mu = pm.tile([1, R], F32, tag="mu", name="mu")
                    nc.scalar.activation(mu[:], sum_ps[:], AF.Copy, scale=1.0 / E)
                    ex2 = pm.tile([1, R], F32, tag="ex2", name="ex2")
                    nc.scalar.activation(ex2[:], sq_ps[:], AF.Copy, scale=1.0 / E)
                    var = pm.tile([1, R], F32, tag="var", name="var")
                    nc.vector.tensor_tensor(var[:], mu[:], mu[:], ALU.mult)
                    nc.vector.tensor_tensor(var[:], ex2[:], var[:], ALU.subtract)
                    nc.vector.tensor_scalar_add(var[:], var[:], 1e-5)
                    lnv = pm.tile([1, R], F32, tag="lnv", name="lnv")
                    nc.scalar.activation(lnv[:], var[:], AF.Ln)
                    AB = pm.tile([1, 2 * R], F32, tag="AB", name="AB")
                    nc.scalar.activation(AB[:, 0:R], lnv[:], AF.Exp, scale=-0.5)  # rstd
                    nc.vector.tensor_tensor(AB[:, R:2 * R], mu[:], AB[:, 0:R], ALU.mult)
                    bc = pp.tile([P, 2 * R], F32, tag="mm", name="mm")
                    nc.tensor.matmul(bc[:], ones_row[:], AB[:])
                    for k in range(KT):
                        tmp = pm.tile([P, R], F32, tag="lntmp", bufs=3, name="lntmp")
                        nc.vector.tensor_tensor(tmp[:], u_tiles[k][:], bc[:, 0:R], ALU.mult)
                        nc.vector.tensor_tensor(tmp[:], tmp[:], bc[:, R:2 * R], ALU.subtract)
                        nc.vector.tensor_scalar(out_tiles[k][:], tmp[:],
                                                gP[:, k:k + 1], bP[:, k:k + 1],
                                                op0=ALU.mult, op1=ALU.add)

                # --- attention out-proj + residual + LN1 ---
                u1 = [atile() for _ in range(KT)]

                def evac_attn(m, ps):
                    tmp = pm.tile([P, R], F32, tag="aev", bufs=4, name="aev")
                    nc.scalar.activation(tmp[:], ps[:], AF.Identity,
                                         bias=w['boP'][:, m:m + 1], scale=1.0)
                    nc.vector.tensor_tensor(u1[m][:], tmp[:], x_cur[m][:], ALU.add)
                mm_sweep(ins[f'woT{l}'], oT, KT, evac_attn)

                x_mid = [atile() for _ in range(KT)]
                layer_norm(u1, w['g1P'], w['b1P'], x_mid)
                xb_mid = bshadow(x_mid)

                # --- FFN ---
                hT = [btile() for _ in range(KT)]

                def evac_ffn1(m, ps):
                    nc.scalar.activation(hT[m][:], ps[:], AF.Relu,
                                         bias=w['c1P'][:, m:m + 1], scale=1.0)
                mm_sweep(ins[f'w1T{l}'], xb_mid, DFF // P, evac_ffn1)

                u2 = [atile() for _ in range(KT)]

                def evac_ffn2(m, ps):
                    tmp = pm.tile([P, R], F32, tag="fev", bufs=4, name="fev")
                    nc.scalar.activation(tmp[:], ps[:], AF.Identity,
                                         bias=w['c2P'][:, m:m + 1], scale=1.0)
                    nc.vector.tensor_tensor(u2[m][:], tmp[:], x_mid[m][:], ALU.add)
                mm_sweep(ins[f'w2T{l}'], hT, KT, evac_ffn2)

                x_next = [atile() for _ in range(KT)]
                layer_norm(u2, w['g2P'], w['b2P'], x_next)
                x_cur = x_next

            # ---- output head: x row-major via PE transpose (fp32) ----
            xrow = [pbig.tile([P, E], F32, tag="xrow", name="xrow") for _ in range(2)]
            for m in range(KT):
                for c in range(2):
                    tp = pp.tile([P, P], F32, tag="mm", name="mm")
                    nc.tensor.transpose(tp[:], x_cur[m][:, c * P:(c + 1) * P], ident[:])
                    nc.scalar.copy(xrow[c][:, m * P:(m + 1) * P], tp[:])
            for c in range(2):
                nc.sync.dma_start(out=out_x.ap()[c * P:(c + 1) * P, :], in_=xrow[c][:])

            # ---- rec head: z = x @ recW.T + rec_b; log_softmax along classes ----
            xb_fin = bshadow(x_cur)
            for mi in range(2):
                z_sb = pbig.tile([P, E], F32, tag="z_sb", name="z_sb")
                for n in range(E // 512):
                    z_ps = pp.tile([P, 512], F32, tag="mm", name="mm")
                    for k in range(KT):
                        slab = pw.tile([P, 512], BF16, tag="wslab", name="wslab")
                        nc.sync.dma_start(
                            out=slab[:],
                            in_=recWT.ap()[k * P:(k + 1) * P, n * 512:(n + 1) * 512])
                        nc.tensor.matmul(z_ps[:], xb_fin[k][:, mi * P:(mi + 1) * P],
                                         slab[:], start=(k == 0), stop=(k == KT - 1))
                    nc.vector.tensor_tensor(z_sb[:, n * 512:(n + 1) * 512], z_ps[:],
                                            recb_bc[:, n * 512:(n + 1) * 512], ALU.add)
                mx = pm.tile([P, 1], F32, tag="mx", name="mx")
                nc.vector.reduce_max(mx[:], z_sb[:], axis=mybir.AxisListType.X)
                nmx = pm.tile([P, 1], F32, tag="nmx", name="nmx")
                nc.vector.tensor_scalar_mul(nmx[:], mx[:], -1.0)
                scratch = pbig.tile([P, E], F32, tag="scratch", name="scratch")
                sume = pm.tile([P, 1], F32, tag="sume", name="sume")
                nc.scalar.activation(scratch[:], z_sb[:], AF.Exp, bias=nmx[:],
                                     accum_out=sume[:])
                lse = pm.tile([P, 1], F32, tag="lse", name="lse")
                nc.scalar.activation(lse[:], sume[:], AF.Ln)
                shift = pm.tile([P, 1], F32, tag="shift", name="shift")
                nc.vector.tensor_tensor(shift[:], mx[:], lse[:], ALU.add)
                nc.vector.tensor_scalar_sub(z_sb[:], z_sb[:], shift[:])
                nc.sync.dma_start(out=out_rec.ap()[mi * P:(mi + 1) * P, :], in_=z_sb[:])

    nc.compile()
    return nc


def _get_program():
    if 'nc' not in _CACHE:
        _CACHE['nc'] = _build_program()
    return _CACHE['nc']


def _make_in_maps(text_input, graph_input, params):
    text_input = np.asarray(text_input, np.float32)
    graph_input = np.asarray(graph_input, np.float32)
    t = _host_prep(params)
    assert not t['cross'], "even mix count expected: no cross terms"
    shared = {k: t[k] for k in t if k != 'cross'}
    in_maps = []
    for i in range(NCORES):
        rows = slice(i * R, (i + 1) * R)
        m = dict(shared)
        m['tT'] = _bf(text_input[rows].T)
        m['gT'] = _bf(graph_input[rows].T)
        in_maps.append(m)
    return in_maps


def kernel(text_input, graph_input, attention_mask, params):
    from concourse.bass_utils import run_bass_kernel_spmd

    nc = _get_program()
    in_maps = _make_in_maps(text_input, graph_input, params)

    res = run_bass_kernel_spmd(nc, in_maps, core_ids=list(range(NCORES))).results

    x = np.concatenate([res[i]['out_x'] for i in range(NCORES)], axis=0)
    rec = np.concatenate([res[i]['out_rec'] for i in range(NCORES)], axis=0)[:, None, :]
    output = x[:, None, :]

    aliW = np.asarray(params['ali_W'], np.float64)
    alib = np.asarray(params['ali_b'], np.float64)
    za = x.astype(np.float64) @ aliW.T + alib
    za -= za.max(-1, keepdims=True)
    ali = (za - np.log(np.exp(za).sum(-1, keepdims=True))).astype(np.float32)

    return output, ali, rec


def time_device(d, reps=10):
    """Wall-clock the sharded PJRT executable with device-resident inputs;
    returns min time in ns (upper bound on HW exec: includes dispatch)."""
    import time as _time
    import jax
    import concourse.mybir as mybir
    from concourse import bass2jax
    from jax.experimental.shard_map import shard_map
    from jax.sharding import Mesh, NamedSharding, PartitionSpec

    nc = _get_program()
    in_maps = _make_in_maps(d['text_input'], d['graph_input'], d['params'])

    bass2jax.install_neuronx_cc_hook()
    partition_name = nc.partition_id_tensor.name if nc.partition_id_tensor else None
    in_names, out_names, out_avals, zero_outs = [], [], [], []
    for alloc in nc.m.functions[0].allocations:
        if not isinstance(alloc, mybir.MemoryLocationSet):
            continue
        name = alloc.memorylocations[0].name
        if alloc.kind == "ExternalInput":
            if name != partition_name:
                in_names.append(name)
        elif alloc.kind == "ExternalOutput":
            shape = tuple(alloc.tensor_shape)
            dtp = mybir.dt.np(alloc.dtype)
            out_names.append(name)
            out_avals.append(jax.core.ShapedArray(shape, dtp))
            zero_outs.append(np.zeros(shape, dtp))
    n_params = len(in_names)
    n_outs = len(out_avals)
    in_names.extend(out_names)
    if partition_name is not None:
        in_names.append(partition_name)

    def _body(*args):
        operands = list(args)
        if partition_name is not None:
            operands.append(bass2jax.partition_id_tensor())
        outs = bass2jax._bass_exec_p.bind(
            *operands, out_avals=tuple(out_avals), in_names=tuple(in_names),
            out_names=tuple(out_names), lowering_input_output_aliases=(),
            sim_require_finite=True, sim_require_nnan=True, nc=nc)
        return tuple(outs)

    devices = jax.devices()[:NCORES]
    mesh = Mesh(np.asarray(devices), ("core",))
    sharded = jax.jit(
        shard_map(_body, mesh=mesh,
                  in_specs=(PartitionSpec("core"),) * (n_params + n_outs),
                  out_specs=(PartitionSpec("core"),) * n_outs,
                  check_rep=False),
        keep_unused=True)
    per_core = [[np.asarray(m[nm]) for nm in in_names[:n_params]] for m in in_maps]
    concat_in = [np.concatenate([per_core[c][i] for c in range(NCORES)], axis=0)
                 for i in range(n_params)]
    concat_zeros = [np.zeros((NCORES * z.shape[0], *z.shape[1:]), z.dtype)
                    for z in zero_outs]
    sh = NamedSharding(mesh, PartitionSpec("core"))
    dev_in = [jax.device_put(a, sh) for a in concat_in + concat_zeros]
    r = sharded(*dev_in)
    jax.block_until_ready(r)
    times = []
    for _ in range(reps):
        t0 = _time.perf_counter()
        r = sharded(*dev_in)
        jax.block_until_ready(r)
        times.append(_time.perf_counter() - t0)
    del r
    print('timing reps (ms):', [round(t * 1e3, 3) for t in times])
    return min(times) * 1e9


# revision 19
# speedup vs baseline: 199.9218x; 199.9218x over previous
"""Trainium2 Bass kernel for nn_CrossAttFuseMod (8 NeuronCores, SPMD).

Sharding: data-parallel over the batch dim B=2048 (256 rows/core).  The
mix/cross-attention stack is degenerate (per-sample seq len 1 => softmax==1),
so the whole pre-encoder is folded on the host into two linear maps.  The
TransformerEncoder self-attention (seq=B, batch=1) gathers K^T and V across
cores with AllGather; everything else is local.  Activations are kept
feature-major ("transposed", [feat, rows]) on-chip so no matmul ever needs an
on-chip transpose; partition-axis reductions (softmax denom, LayerNorm stats)
are done with ones-vector matmuls on the tensor engine.  ScalarE stays inside
one LUT table set (exp/ln + fillers): rsqrt = exp(-0.5*ln(v)), 1/S = exp(-ln S).

dtypes: heavy matmuls run bf16 x bf16 -> fp32 PSUM (1 cycle/row on the PE and
half the weight DMA); the residual stream, LayerNorm stats, softmax sums and
all outputs stay fp32.  HW-measured 3.8e-3 max relative error vs the fp32
reference.
"""

import numpy as np
import ml_dtypes

BF16NP = ml_dtypes.bfloat16

NCORES = 8
B = 2048
E = 2048          # encoder d_model
R = B // NCORES   # rows per core = 256
P = 128           # SBUF partitions
KT = E // P       # 16 k-tiles over the model dim
HN = 8            # heads
HD = E // HN      # 256 head dim
DFF = 2048
HID = 1024

_CACHE = {}


def _pmajor(v):
    # bias vector [F] -> [128, F//128] with (p, m) = v[m*128 + p]
    v = np.ascontiguousarray(np.asarray(v, np.float32))
    return np.ascontiguousarray(v.reshape(-1, P).T)


def _bf(a):
    return np.ascontiguousarray(np.asarray(a, np.float32).astype(BF16NP))


def _fold_params(params):
    """Fold the (purely linear) pre-encoder into x = [T@maW.T+mac | G@mbG.T+mbc]."""
    f8 = np.float64
    g = lambda a: np.asarray(a, f8)
    A, a = g(params['te1_W']), g(params['te1_b'])
    Bm, b = g(params['ge1_W']), g(params['ge1_b'])
    Wt, Gt, ct = A, np.zeros((HID, E)), a
    Wg, Gg, cg = np.zeros((HID, E)), Bm, b
    for pa, pb in zip(params['mix_a'], params['mix_b']):
        Ma = g(pa['Wo']) @ g(pa['Wv'])
        Mb = g(pb['Wo']) @ g(pb['Wv'])
        ba, bb = g(pa['bo']), g(pb['bo'])
        nWt, nGt, nct = Ma @ Wg, Ma @ Gg, Ma @ cg + ba
        nWg, nGg, ncg = Mb @ Wt, Mb @ Gt, Mb @ ct + bb
        Wt, Gt, ct, Wg, Gg, cg = nWt, nGt, nct, nWg, nGg, ncg
    C, c = g(params['te2_W']), g(params['te2_b'])
    D, d = g(params['ge2_W']), g(params['ge2_b'])
    maW, maG, mac = C @ Wt, C @ Gt, C @ ct + c
    mbW, mbG, mbc = D @ Wg, D @ Gg, D @ cg + d
    return (maW, maG, mac), (mbW, mbG, mbc)


def _host_prep(params):
    """All host-side weight prep; returns dict of np arrays shipped to device."""
    (maW, maG, mac), (mbW, mbG, mbc) = _fold_params(params)
    t = {}
    t['PT'] = _bf(maW.T)   # [2048, 1024]
    t['QT'] = _bf(mbG.T)   # [2048, 1024]
    t['pqbP'] = _pmajor(np.concatenate([mac, mbc]).astype(np.float32))  # [128, 16]
    # with an even mix count, ma depends only on text and mb only on graph
    t['cross'] = bool(np.abs(maG).max() > 0 or np.abs(mbW).max() > 0)
    for l, lp in enumerate(params['enc']):
        t[f'winT{l}'] = _bf(np.asarray(lp['Win'], np.float32).T)  # [2048, 6144]
        t[f'binP{l}'] = _pmajor(lp['bin'])                        # [128, 48]
        t[f'woT{l}'] = _bf(np.asarray(lp['Wo'], np.float32).T)
        t[f'boP{l}'] = _pmajor(lp['bo'])
        t[f'w1T{l}'] = _bf(np.asarray(lp['W1'], np.float32).T)
        t[f'c1P{l}'] = _pmajor(lp['c1'])
        t[f'w2T{l}'] = _bf(np.asarray(lp['W2'], np.float32).T)
        t[f'c2P{l}'] = _pmajor(lp['c2'])
        t[f'g1P{l}'] = _pmajor(lp['g1'])
        t[f'b1P{l}'] = _pmajor(lp['b1'])
        t[f'g2P{l}'] = _pmajor(lp['g2'])
        t[f'b2P{l}'] = _pmajor(lp['b2'])
    t['recWT'] = _bf(np.asarray(params['rec_W'], np.float32).T)   # [2048, 2048]
    t['recb'] = np.ascontiguousarray(np.asarray(params['rec_b'], np.float32).reshape(1, -1))
    return t


def _build_program(n_layers=2, sim=False):
    import concourse.bacc as bacc
    import concourse.mybir as mybir
    import concourse.tile as tile

    dt = mybir.dt
    F32 = dt.float32
    BF16 = dt.bfloat16
    AF = mybir.ActivationFunctionType
    ALU = mybir.AluOpType
    RG = [list(range(NCORES))]

    nc = bacc.Bacc("TRN2", target_bir_lowering=False, debug=False,
                   num_devices=NCORES, name="crossattfuse")

    ins = {}

    def inp(name, shape, dtp=F32):
        ins[name] = nc.dram_tensor(name, list(shape), dtp, kind="ExternalInput")
        return ins[name]

    tT = inp('tT', (E, R), BF16)
    gT = inp('gT', (E, R), BF16)
    PT = inp('PT', (E, HID), BF16)
    QT = inp('QT', (E, HID), BF16)
    inp('pqbP', (P, 16))
    for l in range(n_layers):
        inp(f'winT{l}', (E, 3 * E), BF16)
        inp(f'binP{l}', (P, 48))
        inp(f'woT{l}', (E, E), BF16)
        inp(f'boP{l}', (P, 16))
        inp(f'w1T{l}', (E, DFF), BF16)
        inp(f'c1P{l}', (P, DFF // P))
        inp(f'w2T{l}', (DFF, E), BF16)
        inp(f'c2P{l}', (P, 16))
        inp(f'g1P{l}', (P, 16))
        inp(f'b1P{l}', (P, 16))
        inp(f'g2P{l}', (P, 16))
        inp(f'b2P{l}', (P, 16))
    recWT = inp('recWT', (E, E), BF16)
    recb = inp('recb', (1, E))

    out_x = nc.dram_tensor('out_x', [R, E], F32, kind="ExternalOutput")
    out_rec = nc.dram_tensor('out_rec', [R, E], F32, kind="ExternalOutput")

    ident_dram = nc.inline_tensor(np.eye(P, dtype=np.float32), name="ident128")

    with tile.TileContext(nc) as tc:
        with (
            tc.tile_pool(name="const", bufs=1) as pc,
            tc.tile_pool(name="act", bufs=56) as pa,       # fp32 [128,256] tiles
            tc.tile_pool(name="actb", bufs=56) as pb,      # bf16 [128,256] tiles
            tc.tile_pool(name="wslab", bufs=4) as pw,
            tc.tile_pool(name="misc", bufs=2) as pm,
            tc.tile_pool(name="big", bufs=2) as pbig,
            tc.tile_pool(name="psum", bufs=8, space="PSUM") as pp,
            tc.tile_pool(name="dram", bufs=1, space="DRAM") as pd,
        ):
            def atile():
                return pa.tile([P, R], F32, tag="a", name="a")

            def btile():
                return pb.tile([P, R], BF16, tag="ab", name="ab")

            # ---- constants ----
            ones_col = pc.tile([P, 1], F32, tag="ones_col", name="ones_col")
            nc.vector.memset(ones_col[:], 1.0)
            ones_colb = pc.tile([P, 1], BF16, tag="ones_colb", name="ones_colb")
            nc.vector.memset(ones_colb[:], 1.0)
            ones_row = pc.tile([1, P], F32, tag="ones_row", name="ones_row")
            nc.vector.memset(ones_row[:], 1.0)
            ident = pc.tile([P, P], F32, tag="ident", name="ident")
            nc.sync.dma_start(out=ident[:], in_=ident_dram.ap())

            def load_pm(name, cols):
                tl = pc.tile([P, cols], F32, tag=name)
                nc.sync.dma_start(out=tl[:], in_=ins[name].ap())
                return tl

            pqb = load_pm('pqbP', 16)
            lw = []
            for l in range(n_layers):
                lw.append({k: load_pm(f'{k}{l}', 48 if k == 'binP' else 16)
                           for k in ('binP', 'boP', 'c1P', 'c2P', 'g1P', 'b1P', 'g2P', 'b2P')})

            # rec_b broadcast tile [128, 2048] (k=1 ones matmuls, fp32)
            recb_sb = pc.tile([1, E], F32, tag="recb_sb", name="recb_sb")
            nc.sync.dma_start(out=recb_sb[:], in_=recb.ap())
            recb_bc = pc.tile([P, E], F32, tag="recb_bc", name="recb_bc")
            for n in range(E // 512):
                ps = pp.tile([P, 512], F32, tag="mm", name="mm")
                nc.tensor.matmul(ps[:], ones_row[:], recb_sb[:, n * 512:(n + 1) * 512])
                nc.scalar.copy(recb_bc[:, n * 512:(n + 1) * 512], ps[:])

            # ---- bf16 matmul sweep: out[m] = sum_k Wslab(k,m).T @ rhs[k] ----
            def mm_sweep(wdram, rhs_tiles, n_mt, evac, wcol0=0, group=4):
                n_k = len(rhs_tiles)
                for m0 in range(0, n_mt, group):
                    gsz = min(group, n_mt - m0)
                    psl = [pp.tile([P, R], F32, tag="mm", name="mm") for _ in range(gsz)]
                    for k in range(n_k):
                        slab = pw.tile([P, group * P], BF16, tag="wslab", name="wslab")
                        c0 = wcol0 + m0 * P
                        nc.sync.dma_start(
                            out=slab[:, :gsz * P],
                            in_=wdram.ap()[k * P:(k + 1) * P, c0:c0 + gsz * P])
                        for j in range(gsz):
                            nc.tensor.matmul(
                                psl[j][:], slab[:, j * P:(j + 1) * P], rhs_tiles[k][:],
                                start=(k == 0), stop=(k == n_k - 1))
                    for j in range(gsz):
                        evac(m0 + j, psl[j])

            def bshadow(x_tiles):
                xb = []
                for k in range(KT):
                    xbk = btile()
                    nc.vector.tensor_copy(xbk[:], x_tiles[k][:])
                    xb.append(xbk)
                return xb

            # ---- phase 1: folded pre-encoder ----
            tin = [btile() for _ in range(KT)]
            gin = [btile() for _ in range(KT)]
            for k in range(KT):
                nc.sync.dma_start(out=tin[k][:], in_=tT.ap()[k * P:(k + 1) * P, :])
                nc.sync.dma_start(out=gin[k][:], in_=gT.ap()[k * P:(k + 1) * P, :])

            x_cur = [atile() for _ in range(KT)]

            def evac_x(base):
                def f(m, ps):
                    nc.scalar.activation(x_cur[base + m][:], ps[:], AF.Identity,
                                         bias=pqb[:, base + m:base + m + 1], scale=1.0)
                return f
            mm_sweep(PT, tin, HID // P, evac_x(0))
            mm_sweep(QT, gin, HID // P, evac_x(8))

            # ---- encoder layers ----
            for l in range(n_layers):
                w = lw[l]
                winT = ins[f'winT{l}']
                xb_cur = bshadow(x_cur)
                kb = pd.tile([E, R], BF16, tag=f"kb{l}", name="kb")
                vb = pd.tile([R, E], BF16, tag=f"vb{l}", name="vb")
                agk = pd.tile([NCORES * E, R], BF16, tag=f"agk{l}", name="agk",
                              addr_space="Shared")
                agv = pd.tile([B, E], BF16, tag=f"agv{l}", name="agv",
                              addr_space="Shared")

                # --- K^T (winT cols 2048:4096): evacuate bf16 + bounce, AllGather ---
                def evac_k(m, ps):
                    kt = btile()
                    nc.scalar.activation(kt[:], ps[:], AF.Identity,
                                         bias=w['binP'][:, 16 + m:17 + m], scale=1.0)
                    nc.sync.dma_start(out=kb[m * P:(m + 1) * P, :], in_=kt[:])
                mm_sweep(winT, xb_cur, KT, evac_k, wcol0=E)
                if sim:
                    nc.sync.dma_start(out=agk[0:E, :], in_=kb[:, :])
                else:
                    nc.gpsimd.collective_compute(
                        "AllGather", ALU.bypass, replica_groups=RG,
                        ins=[kb.opt()], outs=[agk.opt()])

                # --- V row-major (no bias; folded into oT evac): ---
                for mi in range(2):
                    for dn in range(E // 512):
                        psv = pp.tile([P, 512], F32, tag="mm", name="mm")
                        for k in range(KT):
                            slab = pw.tile([P, 512], BF16, tag="wslab", name="wslab")
                            nc.sync.dma_start(
                                out=slab[:],
                                in_=winT.ap()[k * P:(k + 1) * P,
                                              2 * E + dn * 512:2 * E + (dn + 1) * 512])
                            nc.tensor.matmul(
                                psv[:], xb_cur[k][:, mi * P:(mi + 1) * P], slab[:],
                                start=(k == 0), stop=(k == KT - 1))
                        vsb = pm.tile([P, 512], BF16, tag="vev", bufs=4, name="vev")
                        nc.scalar.copy(vsb[:], psv[:])
                        nc.sync.dma_start(
                            out=vb[mi * P:(mi + 1) * P, dn * 512:(dn + 1) * 512],
                            in_=vsb[:])
                if sim:
                    nc.sync.dma_start(out=agv[0:R, :], in_=vb[:, :])
                else:
                    nc.gpsimd.collective_compute(
                        "AllGather", ALU.bypass, replica_groups=RG,
                        ins=[vb.opt()], outs=[agv.opt()])

                # --- Q^T (winT cols 0:2048) ---
                qT = [btile() for _ in range(KT)]

                def evac_q(m, ps):
                    nc.scalar.activation(qT[m][:], ps[:], AF.Identity,
                                         bias=w['binP'][:, m:m + 1], scale=1.0)
                mm_sweep(winT, xb_cur, KT, evac_q, wcol0=0)

                # --- attention: 4 waves x 2 heads; scores computed t-major (sT) ---
                oT = [btile() for _ in range(KT)]
                inv_sqrt_hd = float(1.0 / np.sqrt(HD))
                for wv in range(4):
                    h0 = 2 * wv
                    S_ps = pp.tile([1, 512], F32, tag="mm", name="mm")
                    o_ps = [pp.tile([P, R], F32, tag="mm", name="mm") for _ in range(4)]
                    for t in range(KT):
                        tg = t * P
                        rrank, tloc = tg // R, tg % R
                        base = rrank * E + h0 * HD
                        kslab = pbig.tile([P, 512], BF16, tag="kslab", name="kslab")
                        nc.sync.dma_start(
                            out=kslab.rearrange("p (blk t) -> p blk t", blk=4),
                            in_=agk[base:base + 512, tloc:tloc + P]
                                .rearrange("(blk p) t -> p blk t", p=P))
                        vslab = pbig.tile([P, 512], BF16, tag="vslab", name="vslab")
                        nc.sync.dma_start(
                            out=vslab[:],
                            in_=agv[tg:tg + P, h0 * HD:h0 * HD + 512])
                        aT = pbig.tile([P, 512], BF16, tag="aT", name="aT", bufs=3)
                        for hh in range(2):
                            s_ps = pp.tile([P, R], F32, tag="mm", name="mm")
                            for dd in range(2):
                                nc.tensor.matmul(
                                    s_ps[:],
                                    kslab[:, (2 * hh + dd) * P:(2 * hh + dd + 1) * P],
                                    qT[2 * (h0 + hh) + dd][:],
                                    start=(dd == 0), stop=(dd == 1))
                            nc.scalar.activation(aT[:, hh * R:(hh + 1) * R], s_ps[:],
                                                 AF.Exp, scale=inv_sqrt_hd)
                        nc.tensor.matmul(S_ps[:], ones_colb[:], aT[:],
                                         start=(t == 0), stop=(t == KT - 1))
                        for j in range(4):
                            hh = j // 2
                            nc.tensor.matmul(
                                o_ps[j][:], vslab[:, j * P:(j + 1) * P],
                                aT[:, hh * R:(hh + 1) * R],
                                start=(t == 0), stop=(t == KT - 1))
                    # normalize: oT = o_ps * (1/S) + bv   (1/S = exp(-ln(S)))
                    lnS = pm.tile([1, 512], F32, tag="lnS", name="lnS")
                    nc.scalar.activation(lnS[:], S_ps[:], AF.Ln)
                    rS = pm.tile([1, 512], F32, tag="rS", name="rS")
                    nc.scalar.activation(rS[:], lnS[:], AF.Exp, scale=-1.0)
                    bc_ps = pp.tile([P, 512], F32, tag="mm", name="mm")
                    nc.tensor.matmul(bc_ps[:], ones_row[:], rS[:])
                    bc_sb = pm.tile([P, 512], F32, tag="bcS", name="bcS")
                    nc.scalar.copy(bc_sb[:], bc_ps[:])
                    for j in range(4):
                        hh, mi = j // 2, j % 2
                        osc = pm.tile([P, R], F32, tag="osc", bufs=4, name="osc")
                        nc.vector.tensor_tensor(osc[:], o_ps[j][:],
                                                bc_sb[:, hh * R:(hh + 1) * R], ALU.mult)
                        nc.scalar.activation(
                            oT[2 * (h0 + hh) + mi][:], osc[:], AF.Identity,
                            bias=w['binP'][:, 32 + 2 * (h0 + hh) + mi:
                                           33 + 2 * (h0 + hh) + mi], scale=1.0)

                # --- LayerNorm (partition-axis stats via fp32 ones-matmuls) ---
                def layer_norm(u_tiles, gP, bP, out_tiles):
                    sum_ps = pp.tile([1, R], F32, tag="mm", name="mm")
                    sq_ps = pp.tile([1, R], F32, tag="mm", name="mm")
                    for k in range(KT):
                        nc.tensor.matmul(sum_ps[:], ones_col[:], u_tiles[k][:],
                                         start=(k == 0), stop=(k == KT - 1))
                        sq = pm.tile([P, R], F32, tag="sq", bufs=3, name="sq")
                        nc.vector.tensor_tensor(sq[:], u_tiles[k][:], u_tiles[k][:],
                                                ALU.mult)
                        nc.tensor.matmul(sq_ps[:], ones_col[:], sq[:],
                                         start=(k == 0), stop=(k == KT - 1))
                    mu = pm.tile([1, R], F32, tag="mu", name="mu")
                    nc.scalar.activation(mu[:], sum_ps[:], AF.Copy, scale=1.0 / E)
                    ex2 = pm.tile([1, R], F32, tag="ex2", name="ex2")
                    nc.scalar.activation(ex2[:], sq_ps[:], AF.Copy, scale=1.0 / E)
                    var = pm.tile([1, R], F32, tag="var", name="var")
                    nc.vector.tensor_tensor(var[:], mu[:], mu[:], ALU.mult)
                    nc.vector.tensor_tensor(var[:], ex2[:], var[:], ALU.subtract)
                    nc.vector.tensor_scalar_add(var[:], var[:], 1e-5)
                    lnv = pm.tile([1, R], F32, tag="lnv", name="lnv")
                    nc.scalar.activation(lnv[:], var[:], AF.Ln)
                    AB = pm.tile([1, 2 * R], F32, tag="AB", name="AB")
                    nc.scalar.activation(AB[:, 0:R], lnv[:], AF.Exp, scale=-0.5)  # rstd
                    nc.vector.tensor_tensor(AB[:, R:2 * R], mu[:], AB[:, 0:R], ALU.mult)
                    bc = pp.tile([P, 2 * R], F32, tag="mm", name="mm")
                    nc.tensor.matmul(bc[:], ones_row[:], AB[:])
                    for k in range(KT):
                        tmp = pm.tile([P, R], F32, tag="lntmp", bufs=3, name="lntmp")
                        nc.vector.tensor_tensor(tmp[:], u_tiles[k][:], bc[:, 0:R], ALU.mult)
                        nc.vector.tensor_tensor(tmp[:], tmp[:], bc[:, R:2 * R], ALU.subtract)
                        nc.vector.tensor_scalar(out_tiles[k][:], tmp[:],
                                                gP[:, k:k + 1], bP[:, k:k + 1],
                                                op0=ALU.mult, op1=ALU.add)

                # --- attention out-proj + residual + LN1 ---
                u1 = [atile() for _ in range(KT)]

                def evac_attn(m, ps):
                    tmp = pm.tile([P, R], F32, tag="aev", bufs=4, name="aev")
                    nc.scalar.activation(tmp[:], ps[:], AF.Identity,
                                         bias=w['boP'][:, m:m + 1], scale=1.0)
                    nc.vector.tensor_tensor(u1[m][:], tmp[:], x_cur[m][:], ALU.add)
                mm_sweep(ins[f'woT{l}'], oT, KT, evac_attn)

                x_mid = [atile() for _ in range(KT)]
                layer_norm(u1, w['g1P'], w['b1P'], x_mid)
                xb_mid = bshadow(x_mid)

                # --- FFN ---
                hT = [btile() for _ in range(KT)]

                def evac_ffn1(m, ps):
                    nc.scalar.activation(hT[m][:], ps[:], AF.Relu,
                                         bias=w['c1P'][:, m:m + 1], scale=1.0)
                mm_sweep(ins[f'w1T{l}'], xb_mid, DFF // P, evac_ffn1)

                u2 = [atile() for _ in range(KT)]

                def evac_ffn2(m, ps):
                    tmp = pm.tile([P, R], F32, tag="fev", bufs=4, name="fev")
                    nc.scalar.activation(tmp[:], ps[:], AF.Identity,
                                         bias=w['c2P'][:, m:m + 1], scale=1.0)
                    nc.vector.tensor_tensor(u2[m][:], tmp[:], x_mid[m][:], ALU.add)
                mm_sweep(ins[f'w2T{l}'], hT, KT, evac_ffn2)

                x_next = [atile() for _ in range(KT)]
                layer_norm(u2, w['g2P'], w['b2P'], x_next)
                x_cur = x_next

            # ---- output head: x row-major via PE transpose (fp32) ----
            xrow = [pbig.tile([P, E], F32, tag="xrow", name="xrow") for _ in range(2)]
            for m in range(KT):
                for c in range(2):
                    tp = pp.tile([P, P], F32, tag="mm", name="mm")
                    nc.tensor.transpose(tp[:], x_cur[m][:, c * P:(c + 1) * P], ident[:])
                    nc.scalar.copy(xrow[c][:, m * P:(m + 1) * P], tp[:])
            for c in range(2):
                nc.sync.dma_start(out=out_x.ap()[c * P:(c + 1) * P, :], in_=xrow[c][:])

            # ---- rec head: z = x @ recW.T + rec_b; log_softmax along classes ----
            xb_fin = bshadow(x_cur)
            for mi in range(2):
                z_sb = pbig.tile([P, E], F32, tag="z_sb", name="z_sb")
                for n in range(E // 512):
                    z_ps = pp.tile([P, 512], F32, tag="mm", name="mm")
                    for k in range(KT):
                        slab = pw.tile([P, 512], BF16, tag="wslab", name="wslab")
                        nc.sync.dma_start(
                            out=slab[:],
                            in_=recWT.ap()[k * P:(k + 1) * P, n * 512:(n + 1) * 512])
                        nc.tensor.matmul(z_ps[:], xb_fin[k][:, mi * P:(mi + 1) * P],
                                         slab[:], start=(k == 0), stop=(k == KT - 1))
                    nc.vector.tensor_tensor(z_sb[:, n * 512:(n + 1) * 512], z_ps[:],
                                            recb_bc[:, n * 512:(n + 1) * 512], ALU.add)
                mx = pm.tile([P, 1], F32, tag="mx", name="mx")
                nc.vector.reduce_max(mx[:], z_sb[:], axis=mybir.AxisListType.X)
                nmx = pm.tile([P, 1], F32, tag="nmx", name="nmx")
                nc.vector.tensor_scalar_mul(nmx[:], mx[:], -1.0)
                scratch = pbig.tile([P, E], F32, tag="scratch", name="scratch")
                sume = pm.tile([P, 1], F32, tag="sume", name="sume")
                nc.scalar.activation(scratch[:], z_sb[:], AF.Exp, bias=nmx[:],
                                     accum_out=sume[:])
                lse = pm.tile([P, 1], F32, tag="lse", name="lse")
                nc.scalar.activation(lse[:], sume[:], AF.Ln)
                shift = pm.tile([P, 1], F32, tag="shift", name="shift")
                nc.vector.tensor_tensor(shift[:], mx[:], lse[:], ALU.add)
                nc.vector.tensor_scalar_sub(z_sb[:], z_sb[:], shift[:])
                nc.sync.dma_start(out=out_rec.ap()[mi * P:(mi + 1) * P, :], in_=z_sb[:])

    nc.compile()
    return nc


def _get_program():
    if 'nc' not in _CACHE:
        _CACHE['nc'] = _build_program()
    return _CACHE['nc']


def _make_in_maps(text_input, graph_input, params):
    text_input = np.asarray(text_input, np.float32)
    graph_input = np.asarray(graph_input, np.float32)
    t = _host_prep(params)
    assert not t['cross'], "even mix count expected: no cross terms"
    shared = {k: t[k] for k in t if k != 'cross'}
    in_maps = []
    for i in range(NCORES):
        rows = slice(i * R, (i + 1) * R)
        m = dict(shared)
        m['tT'] = _bf(text_input[rows].T)
        m['gT'] = _bf(graph_input[rows].T)
        in_maps.append(m)
    return in_maps


def kernel(text_input, graph_input, attention_mask, params):
    from concourse.bass_utils import run_bass_kernel_spmd

    nc = _get_program()
    in_maps = _make_in_maps(text_input, graph_input, params)

    res = run_bass_kernel_spmd(nc, in_maps, core_ids=list(range(NCORES))).results

    x = np.concatenate([res[i]['out_x'] for i in range(NCORES)], axis=0)
    rec = np.concatenate([res[i]['out_rec'] for i in range(NCORES)], axis=0)[:, None, :]
    output = x[:, None, :]

    aliW = np.asarray(params['ali_W'], np.float64)
    alib = np.asarray(params['ali_b'], np.float64)
    za = x.astype(np.float64) @ aliW.T + alib
    za -= za.max(-1, keepdims=True)
    ali = (za - np.log(np.exp(za).sum(-1, keepdims=True))).astype(np.float32)

    return output, ali, rec


def _pjrt_exec_timer(nc, in_maps, reps=10):
    import time as _time
    import jax
    import concourse.mybir as mybir
    from concourse import bass2jax
    from jax.experimental.shard_map import shard_map
    from jax.sharding import Mesh, NamedSharding, PartitionSpec

    bass2jax.install_neuronx_cc_hook()
    partition_name = nc.partition_id_tensor.name if nc.partition_id_tensor else None
    in_names, out_names, out_avals, zero_outs = [], [], [], []
    for alloc in nc.m.functions[0].allocations:
        if not isinstance(alloc, mybir.MemoryLocationSet):
            continue
        name = alloc.memorylocations[0].name
        if alloc.kind == "ExternalInput":
            if name != partition_name:
                in_names.append(name)
        elif alloc.kind == "ExternalOutput":
            shape = tuple(alloc.tensor_shape)
            dtp = mybir.dt.np(alloc.dtype)
            out_names.append(name)
            out_avals.append(jax.core.ShapedArray(shape, dtp))
            zero_outs.append(np.zeros(shape, dtp))
    n_params = len(in_names)
    n_outs = len(out_avals)
    in_names.extend(out_names)
    if partition_name is not None:
        in_names.append(partition_name)

    def _body(*args):
        operands = list(args)
        if partition_name is not None:
            operands.append(bass2jax.partition_id_tensor())
        return tuple(bass2jax._bass_exec_p.bind(
            *operands, out_avals=tuple(out_avals), in_names=tuple(in_names),
            out_names=tuple(out_names), lowering_input_output_aliases=(),
            sim_require_finite=True, sim_require_nnan=True, nc=nc))

    devices = jax.devices()[:NCORES]
    mesh = Mesh(np.asarray(devices), ("core",))
    sharded = jax.jit(
        shard_map(_body, mesh=mesh,
                  in_specs=(PartitionSpec("core"),) * (n_params + n_outs),
                  out_specs=(PartitionSpec("core"),) * n_outs,
                  check_rep=False),
        keep_unused=True)
    per_core = [[np.asarray(m[nm]) for nm in in_names[:n_params]] for m in in_maps]
    concat_in = [np.concatenate([per_core[c][i] for c in range(NCORES)], axis=0)
                 for i in range(n_params)]
    concat_zeros = [np.zeros((NCORES * z.shape[0], *z.shape[1:]), z.dtype)
                    for z in zero_outs]
    sh = NamedSharding(mesh, PartitionSpec("core"))
    dev_in = [jax.device_put(a, sh) for a in concat_in + concat_zeros]
    r = sharded(*dev_in)
    jax.block_until_ready(r)
    times = []
    for _ in range(reps):
        t0 = _time.perf_counter()
        r = sharded(*dev_in)
        jax.block_until_ready(r)
        times.append(_time.perf_counter() - t0)
    del r
    return times


def time_device(d, reps=10):
    nc = _get_program()
    in_maps = _make_in_maps(d['text_input'], d['graph_input'], d['params'])
    times = _pjrt_exec_timer(nc, in_maps, reps)
    print('timing reps (ms):', [round(t * 1e3, 3) for t in times])
    return min(times) * 1e9


def _build_trivial():
    import concourse.bacc as bacc
    import concourse.mybir as mybir
    import concourse.tile as tile
    F32 = mybir.dt.float32
    nc = bacc.Bacc("TRN2", target_bir_lowering=False, debug=False,
                   num_devices=NCORES, name="trivial")
    x = nc.dram_tensor('x', [P, P], F32, kind="ExternalInput")
    y = nc.dram_tensor('y', [P, P], F32, kind="ExternalOutput")
    with tile.TileContext(nc) as tc:
        with tc.tile_pool(name="s", bufs=1) as sp:
            t = sp.tile([P, P], F32, tag="t", name="t")
            nc.sync.dma_start(out=t[:], in_=x.ap())
            nc.sync.dma_start(out=y.ap(), in_=t[:])
    nc.compile()
    return nc


def time_floor(reps=10):
    nc = _build_trivial()
    in_maps = [{'x': np.zeros((P, P), np.float32)} for _ in range(NCORES)]
    times = _pjrt_exec_timer(nc, in_maps, reps)
    print('floor reps (ms):', [round(t * 1e3, 3) for t in times])
    return min(times) * 1e9
